# revision 2
# baseline (speedup 1.0000x reference)
"""GCN layer (GCNConv + BatchNorm + ReLU + residual) as a multi-core TRN2 Bass kernel.

Sharding: nodes are range-partitioned across 8 cores (dst-owned edges per core).

Round-based gather-accumulate design: in-edges of each dst node are split into
"rounds" (edge j of a node belongs to round j).  For round r, every dst slot
of the shard gathers y[src] of its r-th in-edge (or a zero row) with one
dma_gather call whose output layout equals h's SBUF layout, so accumulation is
a single full-shard DVE add per round.  No scatter primitive is needed.

Per core:
  xw = x_shard @ W                      (TensorE, fp32)
  y  = dinv * xw                        (dinv = rsqrt(indeg+1); deg counted on host)
  AllGather(y, bf16) -> y_ext[1:N+1]    (rows 0 and N+1 stay zero)
  h = y_own; for r in rounds: h += gather(y_ext, idx_lo[r]) + gather(y_ext, idx_hi[r])
  h *= dinv                             (dst-side normalization)
  stats = [sum(h), sum(h^2)]            (ones-vector matmuls), AllReduce(256 floats)
  out = relu((h - mean) * rsqrt(var+eps) * gamma + beta) + x_shard

The gather index is int16, so y_ext row space is split at LB=32768: "lo" calls
index rows [0, LB) (zero row at 0), "hi" calls index rows [LB-1, N+2) (zero row
at N+1).  Each call covers every dst slot; slots without an edge in that round
(or whose src falls in the other half) gather a zero row.

The bias b of the GCNConv cancels exactly under training-mode BatchNorm
(a per-feature constant shift moves the batch mean by the same constant), so
it is not applied on-chip.
"""

import numpy as np
import ml_dtypes

import concourse.bass as bass
import concourse.bacc as bacc
import concourse.mybir as mybir
import concourse.tile as tile

P = 128
BN_EPS = 1e-5


def cdiv(a, b):
    return -(-a // b)


class Plan:
    pass


# ---------------------------------------------------------------------------
# Host-side preprocessing: pure index manipulation (sharding / layout).
# ---------------------------------------------------------------------------

def preprocess(x, W, gamma, beta, edge_index, n_cores=8, lo_rows=32768,
               idx_rpc=8):
    x = np.ascontiguousarray(np.asarray(x), dtype=np.float32)
    W = np.ascontiguousarray(np.asarray(W), dtype=np.float32)
    gamma = np.asarray(gamma, dtype=np.float32).reshape(1, -1)
    beta = np.asarray(beta, dtype=np.float32).reshape(1, -1)
    ei = np.asarray(edge_index)
    src = ei[0].astype(np.int64)
    dst = ei[1].astype(np.int64)

    N, D = x.shape
    assert D == P
    assert N % n_cores == 0
    SHARD = N // n_cores
    NW = cdiv(SHARD, P)
    PADN = NW * P
    LB = lo_rows                      # lo gather view = y_ext[0:LB]
    NTOT = (SHARD + 1) * n_cores      # AG output rows (leading zero row per core)
    c_z = cdiv(LB - 1, SHARD + 1)     # a core-boundary zero row inside the hi view
    assert c_z < n_cores and NTOT - LB <= 32766 and LB <= 32768

    deg = np.bincount(dst, minlength=N).astype(np.float32) + 1.0

    # per-core edge lists with round index (position among the dst's in-edges)
    order = np.argsort(dst, kind="stable")
    ds = dst[order]
    ss = src[order]
    r_all = np.arange(len(ds)) - np.searchsorted(ds, ds)  # round of each edge
    R = int(r_all.max()) + 1 if len(ds) else 1

    core_of = ds // SHARD
    dloc = ds - core_of * SHARD

    zero_hi = c_z * (SHARD + 1) - (LB - 1)
    plan = Plan()
    plan.n_cores = n_cores
    plan.N, plan.D, plan.SHARD, plan.NW, plan.PADN = N, D, SHARD, NW, PADN
    plan.NTOT = NTOT
    plan.LB, plan.zero_hi = LB, zero_hi
    plan.R = R
    plan.idx_rpc = idx_rpc
    plan.idx_block = PADN // 16       # idx columns per (round, half) block
    plan.idx_cols = R * 2 * plan.idx_block

    in_maps = []
    for r in range(n_cores):
        xs = x[r * SHARD:(r + 1) * SHARD]
        xT = np.zeros((P, PADN), np.float32)
        xT[:, :SHARD] = xs.T
        xNM = np.zeros((PADN, D), np.float32)
        xNM[:SHARD] = xs
        dg = np.ones(PADN, np.float32)
        dg[:SHARD] = deg[r * SHARD:(r + 1) * SHARD]
        degT = np.ascontiguousarray(dg.reshape(NW, P).T)

        m = core_of == r
        er, ed, es = r_all[m], dloc[m], ss[m]
        lo_idx = np.zeros((R, PADN), np.int16)          # zero row of lo view
        hi_idx = np.full((R, PADN), zero_hi, np.int16)  # zero row of hi view
        row = es + es // SHARD + 1                      # y_ext row of src
        ml = row <= LB - 1
        lo_idx[er[ml], ed[ml]] = row[ml].astype(np.int16)
        mh = ~ml
        hi_idx[er[mh], ed[mh]] = (row[mh] - (LB - 1)).astype(np.int16)

        # idx matrix: per round, lo block then hi block; idx i of a call at
        # [i % 16, c0 + i // 16]; replicated across the 8 gpsimd cores.
        blocks = np.empty((R * 2, 16, plan.idx_block), np.int16)
        blocks[0::2] = lo_idx.reshape(R, plan.idx_block, 16).transpose(0, 2, 1)
        blocks[1::2] = hi_idx.reshape(R, plan.idx_block, 16).transpose(0, 2, 1)
        idx16 = np.concatenate(list(blocks), axis=1)
        idxT = np.tile(idx16, (8, 1))

        in_maps.append({
            "xT": xT,
            "xNM": xNM,
            "Wm": W,
            "gam": gamma,
            "bet": beta,
            "degT": degT,
            "idxT": idxT,
        })
    return plan, in_maps


# ---------------------------------------------------------------------------
# Bass program (SPMD, one program for all cores)
# ---------------------------------------------------------------------------

def build_nc(plan, reps=1, no_coll=False, no_gather=False):
    dt = mybir.dt
    f32, b16, i16 = dt.float32, dt.bfloat16, dt.int16
    NW, PADN, SHARD, D, N, R = plan.NW, plan.PADN, plan.SHARD, plan.D, plan.N, plan.R
    LB = plan.LB
    FULLW = SHARD // P
    REM = SHARD - FULLW * P
    NG = cdiv(NW, 4)                  # 4-window groups for xw
    SG = cdiv(NW, 4)                  # stats groups (N=512)
    STATN = SG * 4 * P                # padded column count for stats
    RPC = plan.idx_rpc
    IBLK = plan.idx_block
    rg = [list(range(plan.n_cores))]

    nc = bacc.Bacc("TRN2", target_bir_lowering=False, debug=False,
                   num_devices=plan.n_cores)

    xT = nc.dram_tensor("xT", [P, PADN], f32, kind="ExternalInput")
    xNM = nc.dram_tensor("xNM", [PADN, D], f32, kind="ExternalInput")
    Wm = nc.dram_tensor("Wm", [D, D], f32, kind="ExternalInput")
    gam = nc.dram_tensor("gam", [1, D], f32, kind="ExternalInput")
    bet = nc.dram_tensor("bet", [1, D], f32, kind="ExternalInput")
    degT = nc.dram_tensor("degT", [P, NW], f32, kind="ExternalInput")
    idxT = nc.dram_tensor("idxT", [P, plan.idx_cols], i16, kind="ExternalInput")
    outT = nc.dram_tensor("outT", [SHARD, D], f32, kind="ExternalOutput")

    with tile.TileContext(nc) as tc:
        with (
            tc.tile_pool(name="const", bufs=1) as cpool,
            tc.tile_pool(name="big", bufs=1) as big,
            tc.tile_pool(name="xp", bufs=1) as xpool,
            tc.tile_pool(name="dram", bufs=1, space="DRAM") as dram,
            tc.tile_pool(name="gbuf", bufs=2) as gpool,
            tc.tile_pool(name="ibuf", bufs=2) as ipool,
            tc.tile_pool(name="sqp", bufs=2) as sqpool,
            tc.tile_pool(name="pxw", bufs=2, space="PSUM") as pxw_pool,
            tc.tile_pool(name="pst", bufs=1, space="PSUM") as pst_pool,
        ):
            w_sb = cpool.tile([P, D], f32)
            deg_sb = cpool.tile([P, NW], f32)
            dinv_sb = cpool.tile([P, NW], f32)
            gam_sb = cpool.tile([1, D], f32)
            bet_sb = cpool.tile([1, D], f32)
            ones_col = cpool.tile([P, 1], f32)
            ones_row = cpool.tile([1, P], f32)
            zrow = cpool.tile([1, D], b16)
            stats_sb = cpool.tile([1, 4 * D], f32)
            arstats_sb = cpool.tile([1, 2 * D], f32)
            mean_sb = cpool.tile([1, D], f32)
            ex2_sb = cpool.tile([1, D], f32)
            var_sb = cpool.tile([1, D], f32)
            istd_sb = cpool.tile([1, D], f32)
            tmp_row = cpool.tile([1, D], f32)
            rows_sb = cpool.tile([1, 2 * D], f32)
            bc_sb = cpool.tile([P, 2 * D], f32)

            y_sb = big.tile([P, PADN], f32)
            h_sb = big.tile([P, STATN], f32)

            for _rep in range(reps):
                ybf_sb = big.tile([P, PADN], b16, tag="ybf", name="ybf")
                y_shard = dram.tile([SHARD + 1, D], b16, tag="ysh", name="ysh")
                y_ext = dram.tile([plan.NTOT, D], b16, addr_space="Shared",
                                  tag="yext", name="yext")
                stats_in = dram.tile([1, 2 * D], f32, tag="sti", name="sti")
                stats_out = dram.tile([1, 2 * D], f32, addr_space="Shared",
                                      tag="sto", name="sto")

                # ---- input loads ----
                nc.sync.dma_start(out=w_sb[:], in_=Wm.ap())
                nc.sync.dma_start(out=deg_sb[:], in_=degT.ap())
                nc.sync.dma_start(out=gam_sb[:], in_=gam.ap())
                nc.sync.dma_start(out=bet_sb[:], in_=bet.ap())
                x_sb = xpool.tile([P, PADN], f32, tag="x", name="x_sb")
                nc.sync.dma_start(out=x_sb[:], in_=xT.ap())
                nc.vector.memset(ones_col[:], 1.0)
                nc.vector.memset(ones_row[:], 1.0)
                nc.vector.memset(zrow[:], 0.0)
                if STATN > PADN:
                    nc.vector.memset(h_sb[:, PADN:STATN], 0.0)

                nc.scalar.activation(out=dinv_sb[:], in_=deg_sb[:],
                                     func=mybir.ActivationFunctionType.Sqrt)
                nc.vector.reciprocal(out=dinv_sb[:], in_=dinv_sb[:])

                # ---- xw = x @ W ; y = dinv * xw ----
                for g in range(NG):
                    w0 = g * 4
                    w1 = min(w0 + 4, NW)
                    pxw = pxw_pool.tile([P, 4 * D], f32, tag="pxw")
                    for w in range(w0, w1):
                        nc.tensor.matmul(
                            pxw[:, (w - w0) * D:(w - w0 + 1) * D],
                            lhsT=x_sb[:, w * P:(w + 1) * P], rhs=w_sb[:],
                            start=True, stop=True)
                    nc.vector.tensor_tensor(
                        out=y_sb[:, w0 * D:w1 * D].rearrange(
                            "p (w f) -> p w f", f=D),
                        in0=pxw[:, 0:(w1 - w0) * D].rearrange(
                            "p (w f) -> p w f", f=D),
                        in1=dinv_sb[:, w0:w1].to_broadcast([P, w1 - w0, D]),
                        op=mybir.AluOpType.mult)
                nc.vector.tensor_copy(out=ybf_sb[:], in_=y_sb[:])

                # ---- y shard (with leading zero row) to DRAM + AllGather ----
                nc.sync.dma_start(out=y_shard[0:1, :], in_=zrow[:])
                if FULLW:
                    nc.sync.dma_start(
                        out=y_shard[1:1 + FULLW * P, :].rearrange(
                            "(w p) f -> p w f", p=P),
                        in_=ybf_sb[:, 0:FULLW * D].rearrange(
                            "p (w f) -> p w f", f=D))
                if REM:
                    nc.sync.dma_start(
                        out=y_shard[1 + FULLW * P:1 + SHARD, :],
                        in_=ybf_sb[0:REM, FULLW * D:(FULLW + 1) * D])
                if no_coll:
                    nc.sync.dma_start(out=y_ext[0:SHARD + 1, :], in_=y_shard[:])
                else:
                    nc.gpsimd.collective_compute(
                        "AllGather", mybir.AluOpType.bypass, replica_groups=rg,
                        ins=[y_shard.opt()], outs=[y_ext.opt()])

                # ---- rounds: h = y_own + sum_r gather_r ----
                nc.vector.tensor_copy(out=h_sb[:, 0:PADN], in_=y_sb[:])
                y_lo = y_ext[0:min(LB, plan.NTOT), :]
                y_hi = y_ext[LB - 1:plan.NTOT, :]
                h3 = h_sb[:, 0:PADN].rearrange("p (w f) -> p w f", f=D)
                islab = None
                for r in range(R):
                    if r % RPC == 0:
                        islab = ipool.tile([P, RPC * 2 * IBLK], i16, tag="islab")
                        c0 = r * 2 * IBLK
                        c1 = min(plan.idx_cols, c0 + RPC * 2 * IBLK)
                        nc.sync.dma_start(out=islab[:, 0:c1 - c0],
                                          in_=idxT.ap()[:, c0:c1])
                    boff = (r % RPC) * 2 * IBLK
                    for k, yv in ((0, y_lo), (1, y_hi)):
                        ib = islab[:, boff + k * IBLK:boff + (k + 1) * IBLK]
                        buf = gpool.tile([P, NW, D], b16,
                                         tag="lo" if k == 0 else "hi")
                        if not no_gather:
                            nc.gpsimd.dma_gather(
                                buf[:], yv, ib, num_idxs=PADN,
                                num_idxs_reg=PADN, elem_size=D,
                                single_packet=False)
                            nc.vector.tensor_tensor(
                                out=h3, in0=h3, in1=buf[:],
                                op=mybir.AluOpType.add)

                # ---- h *= dinv (dst-side normalization) ----
                nc.vector.tensor_tensor(
                    out=h3, in0=h3,
                    in1=dinv_sb[:].to_broadcast([P, NW, D]),
                    op=mybir.AluOpType.mult)

                # ---- BN statistics + AllReduce ----
                pst_s = pst_pool.tile([1, 4 * D], f32)
                pst_q = pst_pool.tile([1, 4 * D], f32)
                for g in range(SG):
                    c = slice(g * 4 * D, (g + 1) * 4 * D)
                    sq = sqpool.tile([P, 4 * D], f32, tag="sq")
                    nc.vector.tensor_tensor(out=sq[:], in0=h_sb[:, c],
                                            in1=h_sb[:, c],
                                            op=mybir.AluOpType.mult)
                    nc.tensor.matmul(pst_s[0:1, :], lhsT=ones_col[:, 0:1],
                                     rhs=h_sb[:, c], start=(g == 0),
                                     stop=(g == SG - 1))
                    nc.tensor.matmul(pst_q[0:1, :], lhsT=ones_col[:, 0:1],
                                     rhs=sq[:], start=(g == 0),
                                     stop=(g == SG - 1))
                for pst, off in ((pst_s, 0), (pst_q, D)):
                    nc.vector.tensor_copy(out=stats_sb[0:1, 0:4 * D],
                                          in_=pst[0:1, :])
                    nc.vector.tensor_tensor(
                        out=stats_sb[0:1, 0:2 * D], in0=stats_sb[0:1, 0:2 * D],
                        in1=stats_sb[0:1, 2 * D:4 * D], op=mybir.AluOpType.add)
                    nc.vector.tensor_tensor(
                        out=stats_sb[0:1, 0:D], in0=stats_sb[0:1, 0:D],
                        in1=stats_sb[0:1, D:2 * D], op=mybir.AluOpType.add)
                    nc.sync.dma_start(out=stats_in[0:1, off:off + D],
                                      in_=stats_sb[0:1, 0:D])
                if no_coll:
                    nc.sync.dma_start(out=stats_out[:], in_=stats_in[:])
                else:
                    nc.gpsimd.collective_compute(
                        "AllReduce", mybir.AluOpType.add, replica_groups=rg,
                        ins=[stats_in.opt()], outs=[stats_out.opt()])
                nc.sync.dma_start(out=arstats_sb[:], in_=stats_out[:])

                # ---- BN scalars ----
                inv_n = 1.0 / float(N)
                nc.vector.tensor_scalar(out=mean_sb[:], in0=arstats_sb[0:1, 0:D],
                                        scalar1=inv_n, scalar2=None,
                                        op0=mybir.AluOpType.mult)
                nc.vector.tensor_scalar(out=ex2_sb[:], in0=arstats_sb[0:1, D:2 * D],
                                        scalar1=inv_n, scalar2=None,
                                        op0=mybir.AluOpType.mult)
                nc.vector.tensor_tensor(out=var_sb[:], in0=mean_sb[:],
                                        in1=mean_sb[:], op=mybir.AluOpType.mult)
                nc.vector.tensor_tensor(out=var_sb[:], in0=ex2_sb[:],
                                        in1=var_sb[:], op=mybir.AluOpType.subtract)
                nc.vector.tensor_scalar(out=istd_sb[:], in0=var_sb[:],
                                        scalar1=BN_EPS, scalar2=None,
                                        op0=mybir.AluOpType.add)
                nc.scalar.activation(out=istd_sb[:], in_=istd_sb[:],
                                     func=mybir.ActivationFunctionType.Sqrt)
                nc.vector.reciprocal(out=istd_sb[:], in_=istd_sb[:])
                nc.vector.tensor_tensor(out=rows_sb[0:1, 0:D], in0=gam_sb[:],
                                        in1=istd_sb[:], op=mybir.AluOpType.mult)
                nc.vector.tensor_tensor(out=tmp_row[:], in0=mean_sb[:],
                                        in1=rows_sb[0:1, 0:D],
                                        op=mybir.AluOpType.mult)
                nc.vector.tensor_tensor(out=rows_sb[0:1, D:2 * D], in0=bet_sb[:],
                                        in1=tmp_row[:], op=mybir.AluOpType.subtract)
                pbc = pst_pool.tile([P, 2 * D], f32)
                nc.tensor.matmul(pbc[:], lhsT=ones_row[0:1, :],
                                 rhs=rows_sb[0:1, :], start=True, stop=True)
                nc.vector.tensor_copy(out=bc_sb[:], in_=pbc[:])

                # ---- finalize: out = relu(h*scale + shift) + x ----
                xnm_sb = xpool.tile([P, PADN], f32, tag="x", name="xnm_sb")
                nc.sync.dma_start(
                    out=xnm_sb[:].rearrange("p (w f) -> p w f", f=D),
                    in_=xNM.ap().rearrange("(w p) f -> p w f", p=P))
                scale_bc = bass.AP(bc_sb[:].tensor, bc_sb[:].offset,
                                   [bc_sb[:].ap[0], [0, NW], [1, D]])
                shift_bc = bass.AP(bc_sb[:].tensor, bc_sb[:].offset + D,
                                   [bc_sb[:].ap[0], [0, NW], [1, D]])
                nc.vector.tensor_tensor(out=h3, in0=h3, in1=scale_bc,
                                        op=mybir.AluOpType.mult)
                nc.vector.tensor_tensor(out=h3, in0=h3, in1=shift_bc,
                                        op=mybir.AluOpType.add)
                nc.scalar.activation(out=h_sb[:, 0:PADN], in_=h_sb[:, 0:PADN],
                                     func=mybir.ActivationFunctionType.Relu)
                nc.vector.tensor_tensor(out=h3, in0=h3,
                                        in1=xnm_sb[:].rearrange(
                                            "p (w f) -> p w f", f=D),
                                        op=mybir.AluOpType.add)
                if FULLW:
                    nc.sync.dma_start(
                        out=outT.ap()[0:FULLW * P, :].rearrange(
                            "(w p) f -> p w f", p=P),
                        in_=h_sb[:, 0:FULLW * D].rearrange(
                            "p (w f) -> p w f", f=D))
                if REM:
                    nc.sync.dma_start(
                        out=outT.ap()[FULLW * P:SHARD, :],
                        in_=h_sb[0:REM, FULLW * D:(FULLW + 1) * D])

    nc.compile()
    return nc


# ---------------------------------------------------------------------------
# Entry point: full inputs in, full output out.
# ---------------------------------------------------------------------------

_CACHE = {}


def kernel(x, W, b, gamma, beta, edge_index):
    from concourse import bass_utils
    plan, in_maps = preprocess(x, W, gamma, beta, edge_index, n_cores=8)
    key = (plan.N, plan.D, plan.R)
    nc = _CACHE.get(key)
    if nc is None:
        nc = build_nc(plan)
        _CACHE[key] = nc
    res = None
    for attempt in range(3):
        try:
            res = bass_utils.run_bass_kernel_spmd(
                nc, in_maps, core_ids=list(range(plan.n_cores)))
            break
        except Exception:  # a wedged device usually recovers on retry
            if attempt == 2:
                raise
    outs = [r["outT"] for r in res.results]
    return np.ascontiguousarray(np.concatenate(outs, axis=0), dtype=np.float32)
